# revision 2
# baseline (speedup 1.0000x reference)
"""Dilated segment attention on 8 TRN2 NeuronCores (Bass/Tile).

Problem (hardcoded from spec):
  x [2, 8192, 2048] f32, Wqkv [6144, 2048], b_qkv [6144], Wout [2048, 2048],
  b_out [2048].  segment=512, dilation=2 -> 16 segments of L=256 dilated
  tokens per batch; per-segment 16-head attention (hd=128); fused qkv and
  out projections.  Output [2, 4096, 2048] f32.

Sharding: the 32 (batch, segment) instances are independent -> 4 per core.
Host pre-gathers the dilated tokens, pre-transposes/pre-tiles operands and
casts to bf16 (compute precision; measured end-to-end rel err ~5e-3).

Per-core dataflow (all matmuls K=128, bf16):
  qkv proj   : feature-major  qkvT[e, tok] = W-tile.T @ xsT-tile  (accum 16 d-tiles)
  scores     : scoresT[lk, lq] = kT.T @ qT  (per seg, head; operands swapped)
  softmax    : exp on ScalarE (scale=1/sqrt(hd); scores provably in [-6, 6]
               so no max subtraction), sums via ones-matmul, normalize on DVE
  AV         : outT[hd, lq] = v[lk, hd].T @ expT[lk, lq]
  out proj   : out[l, e] = aT-tile.T @ WoutT-tile  (accum 16 head-tiles,
               token-major, so the HBM store is linear)
b_out is applied on the host (purely linear post-op); b_qkv is applied
on-chip since it feeds the softmax nonlinearity.

Schedule (v2 — trace-driven):
  - xst is 4 independent quarter tiles so the first chunk's matmuls start
    as soon as quarter 0 lands instead of waiting for the whole 4MB.
  - Initial DMAs are split across the two HWDGE rings (sync: first W chunk
    + xst q0; scalar: bq + xst q1-3) so dispatch and transfer overlap.
  - ~12 zero matmuls at kernel start (overlapping the input DMA wait) keep
    the PE HAM busy-window warm so real matmuls run at 2.4GHz immediately.
  - v->vtok xbar transposes are dispatched from the scalar ring: on the
    sync ring they queued behind W-prefetch slot-waits (head-of-line), a
    measured 8us cascade that stalled the PE and re-throttled HAM.
  - psum->sbuf drains are split ScalarE (half0, bias via ACT) / DVE (half1,
    bias via tensor_scalar_add with a per-partition AP) so chunk-boundary
    psum WAR frees twice as fast; out-proj drains alternate engines.
  - attention for head h-1 is interleaved between head h's projection
    matmuls (1 step per 3 d-tiles): the exp ACT latency and the per-tile
    LDWEIGHTS hide under 216ns projection matmuls instead of stalling the
    small-N attention matmuls.  Head 15's attention interleaves into the
    first out-projection psum groups the same way.
"""

import numpy as np
import ml_dtypes

B = 2
S = 8192
D = 2048
H = 16
HD = 128
SEGMENT = 512
DIL = 2
NSEG = S // SEGMENT          # 16
L = SEGMENT // DIL           # 256 dilated tokens per segment
N_CORES = 8
PAIRS = B * NSEG             # 32 independent (b, n) instances
SPC = PAIRS // N_CORES       # 4 segments per core
TOK = SPC * L                # 1024 tokens per core
DT = D // 128                # 16 contraction tiles
NCHUNK = 3 * D // 128        # 48 qkv feature chunks (16 q, 16 k, 16 v)
SCALE = 1.0 / float(np.sqrt(HD))

_PROGRAM = None


def _build_program():
    import concourse.bass as bass
    import concourse.bacc as bacc
    import concourse.tile as tile
    from concourse import mybir

    BF = mybir.dt.bfloat16
    F32 = mybir.dt.float32
    ts = bass.ts
    IDENT = mybir.ActivationFunctionType.Identity
    EXP = mybir.ActivationFunctionType.Exp

    nc = bacc.Bacc("TRN2", target_bir_lowering=False, debug=False,
                   num_devices=N_CORES)

    xst_d = nc.dram_tensor("xst", [128, DT * TOK], BF, kind="ExternalInput")
    wqkv_d = nc.dram_tensor("wqkv_t", [NCHUNK, 128, DT * 128], BF,
                            kind="ExternalInput")
    wout_d = nc.dram_tensor("wout_t", [4, 128, DT * 512], BF, kind="ExternalInput")
    bq_d = nc.dram_tensor("bq_t", [128, NCHUNK], F32, kind="ExternalInput")
    out_d = nc.dram_tensor("out", [TOK, D], F32, kind="ExternalOutput")

    with tile.TileContext(nc) as tc:
        with (
            tc.tile_pool(name="const", bufs=1) as const_p,
            tc.tile_pool(name="big", bufs=1) as big_p,
            tc.tile_pool(name="wq", bufs=6) as w_p,
            tc.tile_pool(name="qk", bufs=4) as qk_p,
            tc.tile_pool(name="vt", bufs=3) as vt_p,
            tc.tile_pool(name="ex", bufs=4) as ex_p,
            tc.tile_pool(name="st", bufs=2) as st_p,
            tc.tile_pool(name="ou", bufs=3) as ou_p,
            tc.tile_pool(name="pp", bufs=4, space="PSUM") as pp_p,
            tc.tile_pool(name="pa", bufs=2, space="PSUM") as pa_p,
        ):
            ones = const_p.tile([128, 1], BF)
            nc.gpsimd.memset(ones[:], 1.0)
            # PE warmup: the HAM clock gate holds the PE at 1.2GHz until
            # ~3.4us of sustained activity.  Burn that window on zero
            # matmuls while the input DMAs stream, so the first real
            # matmul runs at 2.4GHz.
            warm_sb = const_p.tile([128, 512], BF)
            nc.gpsimd.memset(warm_sb[:], 0.0)
            warm_ps = pp_p.tile([128, 512], F32, tag="pp", name="warm")
            for i in range(12):
                nc.tensor.matmul(warm_ps[:], warm_sb[:, 0:128], warm_sb[:],
                                 start=(i == 0), stop=(i == 11))

            # Initial loads, split across the two HWDGE rings so the
            # dispatches and transfers overlap (SDMA round-robins between
            # rings at packet granularity).
            first_w = w_p.tile([128, DT * 128], BF, tag="w", name="first_w")
            nc.sync.dma_start(out=first_w[:], in_=wqkv_d[32])
            xq_sb = [big_p.tile([128, 4, TOK], BF, name=f"xq{k}")
                     for k in range(4)]
            nc.sync.dma_start(out=xq_sb[0][:], in_=xst_d[:, 0:4 * TOK])
            bq_sb = const_p.tile([128, NCHUNK], F32)
            nc.scalar.dma_start(out=bq_sb[:], in_=bq_d[:])
            for kk in range(1, 4):
                nc.scalar.dma_start(
                    out=xq_sb[kk][:],
                    in_=xst_d[:, 4 * kk * TOK:4 * (kk + 1) * TOK],
                )
            vtok_sb = big_p.tile([128, H, SPC * 2, 128], BF)
            aT_sb = big_p.tile([128, SPC, H, L], BF)

            def proj_steps(c, out_tile, wck=None):
                """qkvT chunk c: out_tile[128, TOK] bf16 = (W chunk).T @ xsT + b.

                Generator: yields after each d-tile's matmul pair; emits the
                split-engine psum drains on exhaustion.
                """
                if wck is None:
                    wck = w_p.tile([128, DT * 128], BF, tag="w")
                    nc.sync.dma_start(out=wck[:], in_=wqkv_d[c])
                pss = [pp_p.tile([128, 512], F32, tag="pp", name=f"ps{half}")
                       for half in range(2)]
                for dt in range(DT):
                    q, r = divmod(dt, 4)
                    for half in range(2):
                        nc.tensor.matmul(
                            pss[half][:],
                            wck[:, ts(dt, 128)],
                            xq_sb[q][:, r, ts(half, 512)],
                            start=(dt == 0),
                            stop=(dt == DT - 1),
                        )
                    yield
                nc.scalar.activation(
                    out=out_tile[:, ts(0, 512)], in_=pss[0][:],
                    func=IDENT, bias=bq_sb[:, c:c + 1], scale=1.0,
                )
                nc.vector.tensor_scalar_add(
                    out_tile[:, ts(1, 512)], pss[1][:], bq_sb[:, c:c + 1],
                )

            def run_gen(g):
                for _ in g:
                    pass

            # ---- v projection (feature-major) + transpose to token-major ----
            # One transposing DMA per head (xbar transpose, ~261GB/s): row
            # tok = tc*128+p of vt.T lands at vtok[p, tc, :], exactly the AV
            # stationary layout.  Dispatched from the scalar HWDGE ring —
            # NOT sync, where W-prefetch slot-waits block it (head-of-line).
            for h in range(H):
                vt_tile = vt_p.tile([128, TOK], BF, tag="vt")
                run_gen(proj_steps(32 + h, vt_tile,
                                   wck=first_w if h == 0 else None))
                nc.scalar.dma_start(out=vtok_sb[:, h, :, :], in_=vt_tile[:],
                                    transpose=True)

            # ---- per-head attention, interleaved into the next head's
            # projection matmul stream ----
            def attention_thunks(h, qh, kh):
                """8 emission steps for head h's attention over 4 segments.

                Step order (sc = scores+exp, av = sums+AV+normalize):
                sc0 sc1 av0 sc2 av1 sc3 av2 av3 — each av(seg) trails its
                exp by >=2 steps (>=6 projection d-tiles ~ 2.6us of PE), so
                the ACT latency is always hidden.
                """
                e_ts = [None] * SPC

                def sc_step(seg):
                    def emit():
                        scT = pa_p.tile([128, 2, L], F32, tag="pa", name="scT")
                        for lkc in range(2):
                            nc.tensor.matmul(
                                scT[:, lkc, :],
                                kh[:, seg * L + lkc * 128:
                                   seg * L + (lkc + 1) * 128],
                                qh[:, seg * L:(seg + 1) * L],
                            )
                        e_t = ex_p.tile([128, 2, L], BF, tag="ex")
                        nc.scalar.activation(out=e_t[:], in_=scT[:],
                                             func=EXP, scale=SCALE)
                        e_ts[seg] = e_t
                    return emit

                def av_step(seg):
                    def emit():
                        e_t = e_ts[seg]
                        # av ([:, 0, :]) and the softmax sums row
                        # ([0:1, 1, :]) share one PSUM bank; Tile
                        # serializes the cross-use.
                        avs = pa_p.tile([128, 2, L], F32, tag="pav", bufs=2,
                                        name="avs")
                        for lkc in range(2):
                            nc.tensor.matmul(
                                avs[0:1, 1, :], ones[:], e_t[:, lkc, :],
                                start=(lkc == 0), stop=(lkc == 1),
                            )
                        for lkc in range(2):
                            nc.tensor.matmul(
                                avs[:, 0, :],
                                vtok_sb[:, h, seg * 2 + lkc, :],
                                e_t[:, lkc, :],
                                start=(lkc == 0), stop=(lkc == 1),
                            )
                        inv = st_p.tile([1, L], F32, tag="st")
                        nc.vector.reciprocal_approx_fast(out=inv[:],
                                                         in_=avs[0:1, 1, :])
                        invB = ex_p.tile([128, L], F32, tag="invb")
                        nc.gpsimd.partition_broadcast(invB[:], inv[:])
                        nc.vector.tensor_mul(aT_sb[:, seg, h, :],
                                             avs[:, 0, :], invB[:])
                    return emit

                return [sc_step(0), sc_step(1), av_step(0), sc_step(2),
                        av_step(1), sc_step(3), av_step(2), av_step(3)]

            def run_interleaved(gens, thunks, every):
                k, ai = 0, 0
                for g in gens:
                    for _ in g:
                        k += 1
                        if k % every == 0 and ai < len(thunks):
                            thunks[ai]()
                            ai += 1
                while ai < len(thunks):
                    thunks[ai]()
                    ai += 1

            wq_eq0 = None
            prev_qk = None
            for h in range(H):
                if h == H - 1:
                    # Prefetch the first Wout quarter one head early so the
                    # out-projection never waits on its 2MB load.
                    wq_eq0 = w_p.tile([128, DT, 512], BF, tag="wo", bufs=2,
                                      name="wq_t")
                    nc.sync.dma_start(out=wq_eq0[:], in_=wout_d[0])
                qh = qk_p.tile([128, TOK], BF, tag="qk")
                kh = qk_p.tile([128, TOK], BF, tag="qk")
                gens = [proj_steps(h, qh), proj_steps(16 + h, kh)]
                thunks = (attention_thunks(h - 1, *prev_qk)
                          if prev_qk is not None else [])
                run_interleaved(gens, thunks, every=3)
                prev_qk = (qh, kh)
            last_attn = attention_thunks(H - 1, *prev_qk)

            # ---- output projection (token-major) ----
            # Wout streamed in four 2MB e-quarters; head 15's attention
            # steps interleave into the first psum groups (its aT d-tile is
            # the last accumulated, so each segment's normalize only has to
            # beat d-tile 15 of its own token tile).
            def po_steps(eq, lc, wq_t):
                seg, lqc = lc // 2, lc % 2
                po = pp_p.tile([128, 512], F32, tag="pp", name="po")
                for dt in range(DT):
                    nc.tensor.matmul(
                        po[:],
                        aT_sb[:, seg, dt, ts(lqc, 128)],
                        wq_t[:, dt, :],
                        start=(dt == 0),
                        stop=(dt == DT - 1),
                    )
                    yield
                ob = ou_p.tile([128, 512], F32, tag="ou")
                if lc % 2:
                    nc.vector.tensor_copy(out=ob[:], in_=po[:])
                else:
                    nc.scalar.activation(out=ob[:], in_=po[:], func=IDENT,
                                         scale=1.0)
                nc.sync.dma_start(
                    out=out_d[lc * 128:(lc + 1) * 128,
                              eq * 512:(eq + 1) * 512],
                    in_=ob[:],
                )

            for eq in range(4):
                if eq == 0:
                    wq_t = wq_eq0
                else:
                    wq_t = w_p.tile([128, DT, 512], BF, tag="wo", bufs=2,
                                    name="wq_t")
                    nc.sync.dma_start(out=wq_t[:], in_=wout_d[eq])
                for lc in range(TOK // 128):
                    thunks = last_attn if (eq == 0 and lc == 0) else []
                    run_interleaved([po_steps(eq, lc, wq_t)], thunks, every=2)

    nc.compile()
    _dedupe_ldweights(nc)
    return nc


def _dedupe_ldweights(nc):
    """Drop InstLdweights whose weights are already resident in the PE array.

    tile_legalize emits one LDWEIGHTS per matmul; consecutive matmuls that
    share the stationary operand (projection token-halves) reload identical
    weights, costing ~97ns of PE pipe each.  Walk each block's PE stream
    tracking the loaded-weights key and delete reloads.  Only semaphore-free
    LDWEIGHTS are dropped, so the sync graph is untouched;
    EVENT_SEMAPHORE/DRAIN between pairs don't disturb the array, any other
    PE instruction conservatively invalidates the key.
    """
    from concourse import mybir

    PE = mybir.EngineType.PE
    dropped = 0
    for f in nc.m.functions:
        for blk in f.blocks:
            insts = blk.instructions
            loaded = None
            to_drop = []
            for idx, x in enumerate(insts):
                if getattr(x, "engine", None) != PE:
                    continue
                nm = type(x).__name__
                if nm == "InstLdweights":
                    si = x.sync_info
                    clean = si is None or (not si.on_wait and not si.on_update)
                    key = (str(x.ins[0]), str(x.is_transpose),
                           str(x.perf_mode), str(x.tile_position))
                    if clean and loaded == key:
                        to_drop.append(idx)
                    else:
                        loaded = key
                elif nm == "InstMatmult":
                    continue
                elif nm in ("InstEventSemaphore", "InstDrain"):
                    continue
                else:
                    loaded = None
            for idx in reversed(to_drop):
                del insts[idx]
            blk.instructions = insts
            dropped += len(to_drop)
    return dropped


def get_program():
    global _PROGRAM
    if _PROGRAM is None:
        _PROGRAM = _build_program()
    return _PROGRAM


def make_in_maps(x, Wqkv, b_qkv):
    """Host-side shard + layout prep (bf16 casts, transposes, tiling)."""
    bf16 = ml_dtypes.bfloat16
    x = np.asarray(x, dtype=np.float32)
    Wqkv = np.asarray(Wqkv, dtype=np.float32)
    b_qkv = np.asarray(b_qkv, dtype=np.float32)

    xs = x.reshape(B, NSEG, SEGMENT, D)[:, :, ::DIL, :]     # [2,16,256,2048]
    xs_flat = xs.reshape(PAIRS, L, D)

    # lhsT tiles packed partition-major: wt[c, p, dt*128+j] = WqkvT[dt*128+p,
    # c*128+j] so one chunk is a single linear per-partition DMA.
    wt = np.ascontiguousarray(
        Wqkv.reshape(NCHUNK, 128, DT, 128).transpose(0, 3, 2, 1)
        .reshape(NCHUNK, 128, DT * 128)
    ).astype(bf16)                                          # [48,128,2048]
    bqt = np.ascontiguousarray(b_qkv.reshape(NCHUNK, 128).T)  # [128,48] f32

    in_maps = []
    for i in range(N_CORES):
        tok = xs_flat[SPC * i:SPC * (i + 1)].reshape(TOK, D)
        xst = np.ascontiguousarray(
            tok.T.reshape(DT, 128, TOK).transpose(1, 0, 2)
            .reshape(128, DT * TOK)).astype(bf16)
        in_maps.append({"xst": xst, "wqkv_t": wt, "bq_t": bqt})
    return in_maps


def make_wout_tiled(Wout):
    Wout = np.asarray(Wout, dtype=np.float32)
    # [eq, p, dt*512+j] = Wout[eq*512+j, dt*128+p]: one linear DMA/quarter
    return np.ascontiguousarray(
        Wout.T.reshape(DT, 128, 4, 512).transpose(2, 1, 0, 3)
        .reshape(4, 128, DT * 512)).astype(ml_dtypes.bfloat16)


def kernel(x, Wqkv, b_qkv, Wout, b_out):
    from concourse import bass_utils

    nc = get_program()
    in_maps = make_in_maps(x, Wqkv, b_qkv)
    wot = make_wout_tiled(Wout)
    for m in in_maps:
        m["wout_t"] = wot

    res = bass_utils.run_bass_kernel_spmd(
        nc, in_maps, core_ids=list(range(N_CORES)))
    outs = [res.results[i]["out"] for i in range(N_CORES)]
    full = np.concatenate(outs, axis=0) + np.asarray(b_out, dtype=np.float32)
    return np.ascontiguousarray(full.reshape(B, NSEG * L, D), dtype=np.float32)


# revision 3
# speedup vs baseline: 1.0468x; 1.0468x over previous
"""Dilated segment attention on 8 TRN2 NeuronCores (Bass/Tile).

Problem (hardcoded from spec):
  x [2, 8192, 2048] f32, Wqkv [6144, 2048], b_qkv [6144], Wout [2048, 2048],
  b_out [2048].  segment=512, dilation=2 -> 16 segments of L=256 dilated
  tokens per batch; per-segment 16-head attention (hd=128); fused qkv and
  out projections.  Output [2, 4096, 2048] f32.

Sharding: the 32 (batch, segment) instances are independent -> 4 per core.
Host pre-gathers the dilated tokens, pre-transposes/pre-tiles operands and
casts to bf16 (compute precision; measured end-to-end rel err ~5e-3).

Per-core dataflow (all matmuls K=128, bf16):
  q/k proj   : feature-major  qkT[e, tok] = W-tile.T @ xsT-tile (accum 16 d-tiles)
  v proj     : token-major    v[tok, e]   = xsT-tile.T @ WvT-quarter
               (drains land directly in the AV-stationary layout — no
               transposes; DMA transposes act as global DMA barriers in
               Tile's deadlock-avoidance and measurably stall the PE)
  scores     : scoresT[lk, lq] = kT.T @ qT  (per seg, head; operands swapped)
  softmax    : exp on ScalarE (scale=1/sqrt(hd); scores provably in [-6, 6]
               so no max subtraction), sums via ones-matmul, normalize on DVE
  AV         : outT[hd, lq] = v[lk, hd].T @ expT[lk, lq]
  out proj   : out[l, e] = aT-tile.T @ WoutT-tile  (accum 16 head-tiles,
               token-major, so the HBM store is linear)
b_qkv's q/k parts are applied on-chip (they feed the softmax); its v part
and b_out are applied on the host: softmax weights sum to 1, so a v bias
shifts the attention output by exactly b_v, i.e. out += Wout @ b_v + b_out.

Schedule (v3 — trace-driven):
  - ~12 zero matmuls at kernel start (overlapping the input DMA wait) keep
    the PE HAM busy-window warm so real matmuls run at 2.4GHz immediately.
  - All startup loads on one HWDGE ring, interleaved [wv0/4, xq0, wv0/4,
    xq1, ...] so the first v-projection group ramps with the arrivals.
  - psum->sbuf drains are split/alternated ScalarE / DVE so chunk-boundary
    psum WAR frees faster and neither engine's FIFO gates the PE.
  - attention for head h-1 is interleaved between head h's projection
    matmuls (1 step per 3 d-tiles): the exp ACT latency and the per-tile
    LDWEIGHTS hide under 216ns projection matmuls.  Head 15's attention
    interleaves into the first out-projection psum groups.
"""

import numpy as np
import ml_dtypes

B = 2
S = 8192
D = 2048
H = 16
HD = 128
SEGMENT = 512
DIL = 2
NSEG = S // SEGMENT          # 16
L = SEGMENT // DIL           # 256 dilated tokens per segment
N_CORES = 8
PAIRS = B * NSEG             # 32 independent (b, n) instances
SPC = PAIRS // N_CORES       # 4 segments per core
TOK = SPC * L                # 1024 tokens per core
DT = D // 128                # 16 contraction tiles
NQK = 2 * D // 128           # 32 q/k feature chunks (16 q, 16 k)
SCALE = 1.0 / float(np.sqrt(HD))

_PROGRAM = None


def _build_program():
    import concourse.bass as bass
    import concourse.bacc as bacc
    import concourse.tile as tile
    from concourse import mybir
    from concourse import bass_isa

    BF = mybir.dt.bfloat16
    F32 = mybir.dt.float32
    ts = bass.ts
    IDENT = mybir.ActivationFunctionType.Identity
    EXP = mybir.ActivationFunctionType.Exp

    nc = bacc.Bacc("TRN2", target_bir_lowering=False, debug=False,
                   num_devices=N_CORES)

    xst_d = nc.dram_tensor("xst", [128, DT * TOK], BF, kind="ExternalInput")
    wqkv_d = nc.dram_tensor("wqkv_t", [NQK, 128, DT * 128], BF,
                            kind="ExternalInput")
    wv_d = nc.dram_tensor("wv_t", [4, 128, DT * 512], BF, kind="ExternalInput")
    wout_d = nc.dram_tensor("wout_t", [4, 128, DT * 512], BF, kind="ExternalInput")
    bq_d = nc.dram_tensor("bq_t", [128, NQK], F32, kind="ExternalInput")
    out_d = nc.dram_tensor("out", [TOK, D], F32, kind="ExternalOutput")

    with tile.TileContext(nc) as tc:
        with (
            tc.tile_pool(name="const", bufs=1) as const_p,
            tc.tile_pool(name="big", bufs=1) as big_p,
            tc.tile_pool(name="wq", bufs=6) as w_p,
            tc.tile_pool(name="qk", bufs=4) as qk_p,
            tc.tile_pool(name="ex", bufs=4) as ex_p,
            tc.tile_pool(name="st", bufs=2) as st_p,
            tc.tile_pool(name="ou", bufs=3) as ou_p,
            tc.tile_pool(name="pp", bufs=4, space="PSUM") as pp_p,
            tc.tile_pool(name="pa", bufs=2, space="PSUM") as pa_p,
        ):
            ones = const_p.tile([128, 1], BF)
            nc.gpsimd.memset(ones[:], 1.0)
            # PE warmup: the HAM clock gate holds the PE at 1.2GHz until
            # ~3.4us of sustained activity.  Burn that window on zero
            # matmuls while the input DMAs stream, so the first real
            # matmul runs at 2.4GHz.
            warm_sb = const_p.tile([128, 512], BF)
            nc.gpsimd.memset(warm_sb[:], 0.0)
            warm_ps = pp_p.tile([128, 512], F32, tag="pp", name="warm")
            for i in range(12):
                nc.tensor.matmul(warm_ps[:], warm_sb[:, 0:128], warm_sb[:],
                                 start=(i == 0), stop=(i == 11))
            # One-off cost probe for a possible future sums offload: a
            # partition_all_reduce on idle GpSimd during the DMA wait.
            par_sb = const_p.tile([128, 512], F32)
            nc.gpsimd.partition_all_reduce(par_sb[:], warm_sb[:], 128,
                                           bass_isa.ReduceOp.add)

            # Startup loads: one FIFO ring, v-quarter-0 slices interleaved
            # with the xst quarters so the first v group's d-tile matmuls
            # start as each (weights, activations) pair lands.
            wv_tiles = [None] * 4
            wv_tiles[0] = w_p.tile([128, DT, 512], BF, tag="wo", bufs=2,
                                   name="wv_t")
            xq_sb = [big_p.tile([128, 4, TOK], BF, name=f"xq{k}")
                     for k in range(4)]
            bq_sb = const_p.tile([128, NQK], F32)
            nc.scalar.dma_start(out=bq_sb[:], in_=bq_d[:])
            for k in range(4):
                nc.sync.dma_start(out=wv_tiles[0][:, 4 * k:4 * (k + 1), :],
                                  in_=wv_d[0][:, 4 * k * 512:4 * (k + 1) * 512])
                nc.sync.dma_start(out=xq_sb[k][:],
                                  in_=xst_d[:, 4 * k * TOK:4 * (k + 1) * TOK])
            vtok_sb = big_p.tile([128, H, SPC * 2, 128], BF)
            aT_sb = big_p.tile([128, SPC, H, L], BF)

            # ---- v projection, token-major (out-proj-style groups) ----
            # vtok[p, h, t, j] = v[tok = t*128+p, hd = j]: each (vc, t)
            # group accumulates [128 tok, 512 v-feats] over the 16 d-tiles
            # and drains straight into the AV-stationary layout.
            for vc in range(4):
                if vc + 1 < 4:
                    wv_tiles[vc + 1] = w_p.tile([128, DT, 512], BF, tag="wo",
                                                bufs=2, name="wv_t")
                    nc.sync.dma_start(out=wv_tiles[vc + 1][:],
                                      in_=wv_d[vc + 1])
                for t in range(SPC * 2):
                    vps = pp_p.tile([128, 512], F32, tag="pp", name="vps")
                    for dt in range(DT):
                        q, r = divmod(dt, 4)
                        nc.tensor.matmul(
                            vps[:],
                            xq_sb[q][:, r, t * 128:(t + 1) * 128],
                            wv_tiles[vc][:, dt, :],
                            start=(dt == 0),
                            stop=(dt == DT - 1),
                        )
                    dst = vtok_sb[:, 4 * vc:4 * (vc + 1), t, :]
                    if t % 2:
                        nc.vector.tensor_copy(out=dst, in_=vps[:])
                    else:
                        nc.scalar.activation(out=dst, in_=vps[:], func=IDENT,
                                             scale=1.0)

            def proj_steps(c, out_tile):
                """q/k chunk c: out_tile[128, TOK] bf16 = (W chunk).T @ xsT + b.

                Generator: yields after each d-tile's matmul pair; emits the
                split-engine psum drains on exhaustion.
                """
                wck = w_p.tile([128, DT * 128], BF, tag="w")
                nc.sync.dma_start(out=wck[:], in_=wqkv_d[c])
                pss = [pp_p.tile([128, 512], F32, tag="pp", name=f"ps{half}")
                       for half in range(2)]
                for dt in range(DT):
                    q, r = divmod(dt, 4)
                    for half in range(2):
                        nc.tensor.matmul(
                            pss[half][:],
                            wck[:, ts(dt, 128)],
                            xq_sb[q][:, r, ts(half, 512)],
                            start=(dt == 0),
                            stop=(dt == DT - 1),
                        )
                    yield
                nc.scalar.activation(
                    out=out_tile[:, ts(0, 512)], in_=pss[0][:],
                    func=IDENT, bias=bq_sb[:, c:c + 1], scale=1.0,
                )
                nc.vector.tensor_scalar_add(
                    out_tile[:, ts(1, 512)], pss[1][:], bq_sb[:, c:c + 1],
                )

            # ---- per-head attention, interleaved into the next head's
            # projection matmul stream ----
            def attention_thunks(h, qh, kh):
                """8 emission steps for head h's attention over 4 segments.

                Step order (sc = scores+exp, av = sums+AV+normalize):
                sc0 sc1 av0 sc2 av1 sc3 av2 av3 — each av(seg) trails its
                exp by >=2 steps (>=6 projection d-tiles ~ 2.6us of PE), so
                the ACT latency is always hidden.
                """
                e_ts = [None] * SPC

                def sc_step(seg):
                    def emit():
                        scT = pa_p.tile([128, 2, L], F32, tag="pa", name="scT")
                        for lkc in range(2):
                            nc.tensor.matmul(
                                scT[:, lkc, :],
                                kh[:, seg * L + lkc * 128:
                                   seg * L + (lkc + 1) * 128],
                                qh[:, seg * L:(seg + 1) * L],
                            )
                        e_t = ex_p.tile([128, 2, L], BF, tag="ex")
                        nc.scalar.activation(out=e_t[:], in_=scT[:],
                                             func=EXP, scale=SCALE)
                        e_ts[seg] = e_t
                    return emit

                def av_step(seg):
                    def emit():
                        e_t = e_ts[seg]
                        # av ([:, 0, :]) and the softmax sums row
                        # ([0:1, 1, :]) share one PSUM bank; Tile
                        # serializes the cross-use.
                        avs = pa_p.tile([128, 2, L], F32, tag="pav", bufs=2,
                                        name="avs")
                        for lkc in range(2):
                            nc.tensor.matmul(
                                avs[0:1, 1, :], ones[:], e_t[:, lkc, :],
                                start=(lkc == 0), stop=(lkc == 1),
                            )
                        for lkc in range(2):
                            nc.tensor.matmul(
                                avs[:, 0, :],
                                vtok_sb[:, h, seg * 2 + lkc, :],
                                e_t[:, lkc, :],
                                start=(lkc == 0), stop=(lkc == 1),
                            )
                        inv = st_p.tile([1, L], F32, tag="st")
                        nc.vector.reciprocal_approx_fast(out=inv[:],
                                                         in_=avs[0:1, 1, :])
                        invB = ex_p.tile([128, L], F32, tag="invb")
                        nc.gpsimd.partition_broadcast(invB[:], inv[:])
                        nc.vector.tensor_mul(aT_sb[:, seg, h, :],
                                             avs[:, 0, :], invB[:])
                    return emit

                return [sc_step(0), sc_step(1), av_step(0), sc_step(2),
                        av_step(1), sc_step(3), av_step(2), av_step(3)]

            def run_interleaved(gens, thunks, every):
                k, ai = 0, 0
                for g in gens:
                    for _ in g:
                        k += 1
                        if k % every == 0 and ai < len(thunks):
                            thunks[ai]()
                            ai += 1
                while ai < len(thunks):
                    thunks[ai]()
                    ai += 1

            wq_eq0 = None
            prev_qk = None
            for h in range(H):
                if h == H - 1:
                    # Prefetch the first Wout quarter one head early so the
                    # out-projection never waits on its 2MB load.
                    wq_eq0 = w_p.tile([128, DT, 512], BF, tag="wo", bufs=2,
                                      name="wq_t")
                    nc.sync.dma_start(out=wq_eq0[:], in_=wout_d[0])
                qh = qk_p.tile([128, TOK], BF, tag="qk")
                kh = qk_p.tile([128, TOK], BF, tag="qk")
                gens = [proj_steps(h, qh), proj_steps(16 + h, kh)]
                thunks = (attention_thunks(h - 1, *prev_qk)
                          if prev_qk is not None else [])
                run_interleaved(gens, thunks, every=3)
                prev_qk = (qh, kh)
            last_attn = attention_thunks(H - 1, *prev_qk)

            # ---- output projection (token-major) ----
            # Wout streamed in four 2MB e-quarters; head 15's attention
            # steps interleave into the first psum group (its aT d-tile is
            # the last accumulated, so each segment's normalize only has to
            # beat d-tile 15 of its own token tile).
            def po_steps(eq, lc, wq_t):
                seg, lqc = lc // 2, lc % 2
                po = pp_p.tile([128, 512], F32, tag="pp", name="po")
                for dt in range(DT):
                    nc.tensor.matmul(
                        po[:],
                        aT_sb[:, seg, dt, ts(lqc, 128)],
                        wq_t[:, dt, :],
                        start=(dt == 0),
                        stop=(dt == DT - 1),
                    )
                    yield
                ob = ou_p.tile([128, 512], F32, tag="ou")
                if lc % 2:
                    nc.vector.tensor_copy(out=ob[:], in_=po[:])
                else:
                    nc.scalar.activation(out=ob[:], in_=po[:], func=IDENT,
                                         scale=1.0)
                nc.sync.dma_start(
                    out=out_d[lc * 128:(lc + 1) * 128,
                              eq * 512:(eq + 1) * 512],
                    in_=ob[:],
                )

            for eq in range(4):
                if eq == 0:
                    wq_t = wq_eq0
                else:
                    wq_t = w_p.tile([128, DT, 512], BF, tag="wo", bufs=2,
                                    name="wq_t")
                    nc.sync.dma_start(out=wq_t[:], in_=wout_d[eq])
                for lc in range(TOK // 128):
                    thunks = last_attn if (eq == 0 and lc == 0) else []
                    run_interleaved([po_steps(eq, lc, wq_t)], thunks, every=2)

    nc.compile()
    _dedupe_ldweights(nc)
    return nc


def _dedupe_ldweights(nc):
    """Drop InstLdweights whose weights are already resident in the PE array.

    tile_legalize emits one LDWEIGHTS per matmul; consecutive matmuls that
    share the stationary operand (projection token-halves) reload identical
    weights, costing ~97ns of PE pipe each.  Walk each block's PE stream
    tracking the loaded-weights key and delete reloads.  Only semaphore-free
    LDWEIGHTS are dropped, so the sync graph is untouched;
    EVENT_SEMAPHORE/DRAIN between pairs don't disturb the array, any other
    PE instruction conservatively invalidates the key.
    """
    from concourse import mybir

    PE = mybir.EngineType.PE
    dropped = 0
    for f in nc.m.functions:
        for blk in f.blocks:
            insts = blk.instructions
            loaded = None
            to_drop = []
            for idx, x in enumerate(insts):
                if getattr(x, "engine", None) != PE:
                    continue
                nm = type(x).__name__
                if nm == "InstLdweights":
                    si = x.sync_info
                    clean = si is None or (not si.on_wait and not si.on_update)
                    key = (str(x.ins[0]), str(x.is_transpose),
                           str(x.perf_mode), str(x.tile_position))
                    if clean and loaded == key:
                        to_drop.append(idx)
                    else:
                        loaded = key
                elif nm == "InstMatmult":
                    continue
                elif nm in ("InstEventSemaphore", "InstDrain"):
                    continue
                else:
                    loaded = None
            for idx in reversed(to_drop):
                del insts[idx]
            blk.instructions = insts
            dropped += len(to_drop)
    return dropped


def get_program():
    global _PROGRAM
    if _PROGRAM is None:
        _PROGRAM = _build_program()
    return _PROGRAM


def make_in_maps(x, Wqkv, b_qkv):
    """Host-side shard + layout prep (bf16 casts, transposes, tiling)."""
    bf16 = ml_dtypes.bfloat16
    x = np.asarray(x, dtype=np.float32)
    Wqkv = np.asarray(Wqkv, dtype=np.float32)
    b_qkv = np.asarray(b_qkv, dtype=np.float32)

    xs = x.reshape(B, NSEG, SEGMENT, D)[:, :, ::DIL, :]     # [2,16,256,2048]
    xs_flat = xs.reshape(PAIRS, L, D)

    # q/k lhsT tiles packed partition-major: wt[c, p, dt*128+j] =
    # WqkvT[dt*128+p, c*128+j] so one chunk is a single linear DMA.
    wt = np.ascontiguousarray(
        Wqkv[:2 * D].reshape(NQK, 128, DT, 128).transpose(0, 3, 2, 1)
        .reshape(NQK, 128, DT * 128)
    ).astype(bf16)                                          # [32,128,2048]
    # v moving quarters, same layout as the Wout quarters.
    wvt = _quarter_tiles(Wqkv[2 * D:])                      # [4,128,DT*512]
    bqt = np.ascontiguousarray(b_qkv[:2 * D].reshape(NQK, 128).T)  # [128,32]

    in_maps = []
    for i in range(N_CORES):
        tok = xs_flat[SPC * i:SPC * (i + 1)].reshape(TOK, D)
        xst = np.ascontiguousarray(
            tok.T.reshape(DT, 128, TOK).transpose(1, 0, 2)
            .reshape(128, DT * TOK)).astype(bf16)
        in_maps.append({"xst": xst, "wqkv_t": wt, "wv_t": wvt, "bq_t": bqt})
    return in_maps


def _quarter_tiles(W):
    """[2048, 2048] row-major -> [eq, p, dt*512+j] = W[eq*512+j, dt*128+p]."""
    W = np.asarray(W, dtype=np.float32)
    return np.ascontiguousarray(
        W.T.reshape(DT, 128, 4, 512).transpose(2, 1, 0, 3)
        .reshape(4, 128, DT * 512)).astype(ml_dtypes.bfloat16)


def make_wout_tiled(Wout):
    return _quarter_tiles(Wout)


def kernel(x, Wqkv, b_qkv, Wout, b_out):
    from concourse import bass_utils

    nc = get_program()
    in_maps = make_in_maps(x, Wqkv, b_qkv)
    wot = make_wout_tiled(Wout)
    for m in in_maps:
        m["wout_t"] = wot

    res = bass_utils.run_bass_kernel_spmd(
        nc, in_maps, core_ids=list(range(N_CORES)))
    outs = [res.results[i]["out"] for i in range(N_CORES)]
    # Softmax weights sum to 1, so the v bias passes through attention
    # unchanged: out += Wout @ b_v + b_out (host-side, exact).
    bias = (np.asarray(Wout, dtype=np.float32)
            @ np.asarray(b_qkv, dtype=np.float32)[2 * D:]
            + np.asarray(b_out, dtype=np.float32))
    full = np.concatenate(outs, axis=0) + bias
    return np.ascontiguousarray(full.reshape(B, NSEG * L, D), dtype=np.float32)
